# revision 1
# baseline (speedup 1.0000x reference)
"""Trainium2 Bass kernel for nn_CognitiveWorkspaceTransformer.

Math (reference semantics):
    X   = S + concat(w_spoke, w_hub_priv, w_hub_shared, tag)   # full 1088 cover
    out = X @ W_read.T          # (B,T,1024)
    k   = latent @ Wk.T         # cache is fully overwritten by latent
    v   = latent @ Wv.T

Sharding: data-parallel over batch B=8, one batch element per NeuronCore.
All tensors are laid out feature-major on the host (pure layout prep, no
arithmetic) so the contraction dim lands on SBUF partitions directly and
the PE needs no on-chip transposes.

Per-core schedule (8 slabs of 512 tokens):
  sync ring:   S^T slab loads (2.2MB), paired-parity output stores
  scalar ring: wcat^T slab loads, weight loads, other-parity output stores
  DVE:         X = S + wcat (per-128-token-group adds), out PSUM->SBUF copies
  ACT:         k/v PSUM->SBUF copies
  PE:          fp32r matmuls (1024-col stationary X^T chunks, 512-col moving
               weight tiles, 9-chunk accumulation over the 1088 contraction)
Measured: ~295-310us on hardware vs a ~262us HBM roofline (91MB/core at
~358GB/s); DMA-active is ~97% of the HBM cap. fp32r (single-pass PE fp32)
gives ~1.9e-4 max relative error vs the fp32 reference.
"""

import numpy as np

import concourse.bacc as bacc
import concourse.mybir as mybir
import concourse.tile as tile
from concourse.bass_utils import run_bass_kernel_spmd

B, T, D_STATE, D_MODEL, D_LATENT = 8, 4096, 1088, 1024, 128
N_CORES = 8
P = 128
F32 = mybir.dt.float32
F32R = mybir.dt.float32r

# feature chunks of the contraction dim (1088 = 8*128 + 64)
R_CHUNKS = [(j * 128, min(128, D_STATE - j * 128)) for j in range((D_STATE + 127) // 128)]
NJ = len(R_CHUNKS)

_NC_CACHE = {}


def build_nc(mm_dt=F32R, t_chunk=512, in_bufs=3, wc_bufs=2, out_bufs=4, mm_bufs=8,
             split_add=True, in_split=1, out_pair=False, taper=False):
    """Build + compile the per-core Bass program (identical on all cores)."""
    if taper:
        slabs = [256] + [512] * ((T - 512) // 512) + [256]
    else:
        slabs = [t_chunk] * (T // t_chunk)
    assert sum(slabs) == T

    nc = bacc.Bacc("TRN2", target_bir_lowering=False, debug=False, num_devices=N_CORES)

    # feature-major inputs: sT/wcT [1088, T], latT [128, T]
    st_d = nc.dram_tensor("st", [D_STATE, T], mm_dt, kind="ExternalInput").ap()
    wct_d = nc.dram_tensor("wct", [D_STATE, T], mm_dt, kind="ExternalInput").ap()
    latt_d = nc.dram_tensor("latt", [D_LATENT, T], mm_dt, kind="ExternalInput").ap()
    wrt_d = nc.dram_tensor("wrt", [D_STATE, D_MODEL], mm_dt, kind="ExternalInput").ap()
    wkt_d = nc.dram_tensor("wkt", [D_LATENT, D_MODEL], mm_dt, kind="ExternalInput").ap()
    wvt_d = nc.dram_tensor("wvt", [D_LATENT, D_MODEL], mm_dt, kind="ExternalInput").ap()
    out_d = nc.dram_tensor("out", [T, D_MODEL], F32, kind="ExternalOutput").ap()
    k_d = nc.dram_tensor("k", [T, D_MODEL], F32, kind="ExternalOutput").ap()
    v_d = nc.dram_tensor("v", [T, D_MODEL], F32, kind="ExternalOutput").ap()

    with tile.TileContext(nc) as tc:
        with (
            tc.tile_pool(name="weights", bufs=1) as wpool,
            tc.tile_pool(name="ins", bufs=in_bufs) as inpool,
            tc.tile_pool(name="wcp", bufs=wc_bufs) as wcpool,
            tc.tile_pool(name="outs", bufs=out_bufs) as outpool,
            tc.tile_pool(name="psum_mm", bufs=mm_bufs, space="PSUM") as mm_pool,
        ):
            # resident weights
            wr_tiles = []
            for j, (r0, rw) in enumerate(R_CHUNKS):
                wt = wpool.tile([rw, D_MODEL], mm_dt, tag=f"wr{j}")
                nc.scalar.dma_start(wt[:], wrt_d[r0 : r0 + rw, :])
                wr_tiles.append(wt)
            wk_t = wpool.tile([D_LATENT, D_MODEL], mm_dt, tag="wk")
            nc.scalar.dma_start(wk_t[:], wkt_d[:])
            wv_t = wpool.tile([D_LATENT, D_MODEL], mm_dt, tag="wv")
            nc.scalar.dma_start(wv_t[:], wvt_d[:])

            t_cursor = 0
            for it, sz in enumerate(slabs):
                t0 = t_cursor
                t_cursor += sz
                ng = sz // P
                # X^T tile: [128 (r within chunk), 9 chunks, t_chunk]
                xt = inpool.tile([P, NJ, sz], mm_dt, tag="x")
                wc = wcpool.tile([P, NJ, sz], mm_dt, tag="wc")
                lt = inpool.tile([P, sz], mm_dt, tag="lt")
                tsl = sz // in_split
                for u in range(in_split):
                    u0 = u * tsl
                    nc.sync.dma_start(
                        xt[:, 0:8, u0 : u0 + tsl],
                        st_d[0:1024, t0 + u0 : t0 + u0 + tsl].rearrange(
                            "(j p) t -> p j t", p=P),
                    )
                    nc.sync.dma_start(
                        xt[0:64, 8, u0 : u0 + tsl],
                        st_d[1024:1088, t0 + u0 : t0 + u0 + tsl])
                    nc.scalar.dma_start(
                        wc[:, 0:8, u0 : u0 + tsl],
                        wct_d[0:1024, t0 + u0 : t0 + u0 + tsl].rearrange(
                            "(j p) t -> p j t", p=P),
                    )
                    nc.scalar.dma_start(
                        wc[0:64, 8, u0 : u0 + tsl],
                        wct_d[1024:1088, t0 + u0 : t0 + u0 + tsl])
                nc.scalar.dma_start(lt[:], latt_d[:, t0 : t0 + sz])
                xr = xt[:]
                if split_add:
                    for g in range(ng):
                        sl = slice(g * P, (g + 1) * P)
                        nc.vector.tensor_add(xr[:, :, sl], xt[:, :, sl], wc[:, :, sl])
                else:
                    nc.vector.tensor_add(xr, xt[:], wc[:])
                ltr = lt[:]

                pair_tiles = {}
                for g in range(ng):
                    ts0 = g * P
                    if out_pair:
                        if g % 2 == 0:
                            out_pr = outpool.tile([P, 2, D_MODEL], F32, tag="out")
                            k_pr = outpool.tile([P, 2, D_MODEL], F32, tag="k")
                            v_pr = outpool.tile([P, 2, D_MODEL], F32, tag="v")
                            pair_tiles = {"out": out_pr, "k": k_pr, "v": v_pr}
                        out_sb = pair_tiles["out"][:, g % 2, :]
                        k_sb = pair_tiles["k"][:, g % 2, :]
                        v_sb = pair_tiles["v"][:, g % 2, :]
                    else:
                        out_sb = outpool.tile([P, D_MODEL], F32, tag="out")
                    for h in range(2):
                        n0 = h * 512
                        po = mm_pool.tile([P, 512], F32, tag="mm")
                        for j, (r0, rw) in enumerate(R_CHUNKS):
                            nc.tensor.matmul(
                                po[:],
                                xr[0:rw, j, ts0 : ts0 + P],
                                wr_tiles[j][0:rw, n0 : n0 + 512],
                                start=(j == 0),
                                stop=(j == NJ - 1),
                            )
                        nc.vector.tensor_copy(out_sb[:, n0 : n0 + 512], po[:])

                    if not out_pair:
                        k_sb = outpool.tile([P, D_MODEL], F32, tag="k")
                        v_sb = outpool.tile([P, D_MODEL], F32, tag="v")
                    for h in range(2):
                        n0 = h * 512
                        pk = mm_pool.tile([P, 512], F32, tag="mm")
                        nc.tensor.matmul(
                            pk[:], ltr[:, ts0 : ts0 + P], wk_t[:, n0 : n0 + 512],
                            start=True, stop=True,
                        )
                        nc.scalar.copy(k_sb[:, n0 : n0 + 512], pk[:])
                        pv = mm_pool.tile([P, 512], F32, tag="mm")
                        nc.tensor.matmul(
                            pv[:], ltr[:, ts0 : ts0 + P], wv_t[:, n0 : n0 + 512],
                            start=True, stop=True,
                        )
                        nc.scalar.copy(v_sb[:, n0 : n0 + 512], pv[:])

                    eng_a, eng_b = (nc.scalar, nc.sync) if g % 2 == 0 else (nc.sync, nc.scalar)
                    if out_pair:
                        if g % 2 == 1:
                            row0 = t0 + ts0 - P
                            dst = lambda ap: ap[row0 : row0 + 2 * P, :].rearrange(
                                "(g p) d -> p g d", p=P)
                            eng_a.dma_start(dst(out_d), pair_tiles["out"][:])
                            eng_b.dma_start(dst(k_d), pair_tiles["k"][:])
                            eng_a.dma_start(dst(v_d), pair_tiles["v"][:])
                    else:
                        row0 = t0 + ts0
                        eng_a.dma_start(out_d[row0 : row0 + P, :], out_sb[:])
                        eng_b.dma_start(k_d[row0 : row0 + P, :], k_sb[:])
                        eng_a.dma_start(v_d[row0 : row0 + P, :], v_sb[:])

    nc.compile()
    return nc


def _get_nc(**kw):
    key = tuple(sorted(kw.items()))
    if key not in _NC_CACHE:
        _NC_CACHE[key] = build_nc(**kw)
    return _NC_CACHE[key]


def make_in_maps(S, w_spoke, w_hub_priv, w_hub_shared, tag, W_read, cache, latent,
                 Wk, Wv):
    # host-side layout prep only (shard over batch, feature-major transposes)
    wcat = np.concatenate(
        [np.asarray(w_spoke, np.float32), np.asarray(w_hub_priv, np.float32),
         np.asarray(w_hub_shared, np.float32), np.asarray(tag, np.float32)],
        axis=-1,
    )
    sT = np.ascontiguousarray(np.asarray(S, np.float32).transpose(0, 2, 1))
    wcT = np.ascontiguousarray(wcat.transpose(0, 2, 1))
    latT = np.ascontiguousarray(np.asarray(latent, np.float32).transpose(0, 2, 1))
    wrt = np.ascontiguousarray(np.asarray(W_read, np.float32).T)
    wkt = np.ascontiguousarray(np.asarray(Wk, np.float32).T)
    wvt = np.ascontiguousarray(np.asarray(Wv, np.float32).T)
    return [
        {"st": sT[i], "wct": wcT[i], "latt": latT[i],
         "wrt": wrt, "wkt": wkt, "wvt": wvt}
        for i in range(N_CORES)
    ]


def kernel(S, w_spoke, w_hub_priv, w_hub_shared, tag, W_read, cache, latent, Wk, Wv,
           **build_kw):
    in_maps = make_in_maps(S, w_spoke, w_hub_priv, w_hub_shared, tag, W_read, cache,
                           latent, Wk, Wv)
    nc = _get_nc(**build_kw)
    res = run_bass_kernel_spmd(nc, in_maps, list(range(N_CORES)))
    out = np.stack([res.results[i]["out"] for i in range(N_CORES)])
    k = np.stack([res.results[i]["k"] for i in range(N_CORES)])
    v = np.stack([res.results[i]["v"] for i in range(N_CORES)])
    return (out, k, v)



# revision 3
# speedup vs baseline: 1.5312x; 1.5312x over previous
"""Trainium2 Bass kernel for nn_CognitiveWorkspaceTransformer.

Math (reference semantics):
    X   = S + concat(w_spoke, w_hub_priv, w_hub_shared, tag)   # full 1088 cover
    out = X @ W_read.T          # (B,T,1024)
    k   = latent @ Wk.T         # cache is fully overwritten by latent
    v   = latent @ Wv.T

Sharding: data-parallel over batch B=8, one batch element per NeuronCore.
All tensors are laid out feature-major on the host (pure layout prep plus a
bf16 downcast, no arithmetic) so the contraction dim lands on SBUF
partitions directly and the PE needs no on-chip transposes.

bf16 everywhere (tolerance is 2e-2; bf16 lands ~3e-3): halves HBM traffic
vs fp32 (was ~91MB/core -> ~262us DMA roofline; now ~47MB/core -> ~130us),
which turns the kernel PE-bound (~360k PE cycles @ 2.4GHz = ~150us/core).

Per-core schedule (slabs of `t_chunk` tokens):
  sync ring:   S^T slab loads, half the output stores
  scalar ring: wcat^T slab loads, weight loads, lat loads, other stores
  DVE:         X = S + wcat adds, out PSUM->SBUF copies (cast to bf16)
  ACT:         k/v PSUM->SBUF copies (cast to bf16)
  PE:          bf16 matmuls; j-outer/h-inner so each 128x128 stationary
               X^T chunk is loaded once and reused for both 512-col halves
"""

import numpy as np
import ml_dtypes

import concourse.bacc as bacc
import concourse.mybir as mybir
import concourse.tile as tile
from concourse.bass_utils import run_bass_kernel_spmd

B, T, D_STATE, D_MODEL, D_LATENT = 8, 4096, 1088, 1024, 128
N_CORES = 8
P = 128
F32 = mybir.dt.float32
BF16 = mybir.dt.bfloat16

# feature chunks of the contraction dim (1088 = 8*128 + 64)
R_CHUNKS = [(j * 128, min(128, D_STATE - j * 128)) for j in range((D_STATE + 127) // 128)]
NJ = len(R_CHUNKS)

_NC_CACHE = {}


def build_nc(mm_dt=BF16, out_dt=BF16, t_chunk=512, in_bufs=3, wc_bufs=2, out_bufs=4,
             mm_bufs=8, split_add=True, in_split=1):
    """Build + compile the per-core Bass program (identical on all cores)."""
    slabs = [t_chunk] * (T // t_chunk)
    assert sum(slabs) == T

    nc = bacc.Bacc("TRN2", target_bir_lowering=False, debug=False, num_devices=N_CORES)

    # feature-major inputs: sT/wcT [1088, T], latT [128, T]
    st_d = nc.dram_tensor("st", [D_STATE, T], mm_dt, kind="ExternalInput").ap()
    wct_d = nc.dram_tensor("wct", [D_STATE, T], mm_dt, kind="ExternalInput").ap()
    latt_d = nc.dram_tensor("latt", [D_LATENT, T], mm_dt, kind="ExternalInput").ap()
    wrt_d = nc.dram_tensor("wrt", [D_STATE, D_MODEL], mm_dt, kind="ExternalInput").ap()
    wkt_d = nc.dram_tensor("wkt", [D_LATENT, D_MODEL], mm_dt, kind="ExternalInput").ap()
    wvt_d = nc.dram_tensor("wvt", [D_LATENT, D_MODEL], mm_dt, kind="ExternalInput").ap()
    out_d = nc.dram_tensor("out", [T, D_MODEL], out_dt, kind="ExternalOutput").ap()
    k_d = nc.dram_tensor("k", [T, D_MODEL], out_dt, kind="ExternalOutput").ap()
    v_d = nc.dram_tensor("v", [T, D_MODEL], out_dt, kind="ExternalOutput").ap()

    with tile.TileContext(nc) as tc:
        with (
            tc.tile_pool(name="weights", bufs=1) as wpool,
            tc.tile_pool(name="ins", bufs=in_bufs) as inpool,
            tc.tile_pool(name="wcp", bufs=wc_bufs) as wcpool,
            tc.tile_pool(name="outs", bufs=out_bufs) as outpool,
            tc.tile_pool(name="psum_mm", bufs=mm_bufs, space="PSUM") as mm_pool,
        ):
            # resident weights
            wr_tiles = []
            for j, (r0, rw) in enumerate(R_CHUNKS):
                wt = wpool.tile([rw, D_MODEL], mm_dt, tag=f"wr{j}")
                nc.scalar.dma_start(wt[:], wrt_d[r0 : r0 + rw, :])
                wr_tiles.append(wt)
            wk_t = wpool.tile([D_LATENT, D_MODEL], mm_dt, tag="wk")
            nc.scalar.dma_start(wk_t[:], wkt_d[:])
            wv_t = wpool.tile([D_LATENT, D_MODEL], mm_dt, tag="wv")
            nc.scalar.dma_start(wv_t[:], wvt_d[:])

            t_cursor = 0
            for it, sz in enumerate(slabs):
                t0 = t_cursor
                t_cursor += sz
                ng = sz // P
                # X^T tile: [128 (r within chunk), 9 chunks, t_chunk]
                xt = inpool.tile([P, NJ, sz], mm_dt, tag="x")
                wc = wcpool.tile([P, NJ, sz], mm_dt, tag="wc")
                lt = inpool.tile([P, sz], mm_dt, tag="lt")
                tsl = sz // in_split
                for u in range(in_split):
                    u0 = u * tsl
                    nc.sync.dma_start(
                        xt[:, 0:8, u0 : u0 + tsl],
                        st_d[0:1024, t0 + u0 : t0 + u0 + tsl].rearrange(
                            "(j p) t -> p j t", p=P),
                    )
                    nc.sync.dma_start(
                        xt[0:64, 8, u0 : u0 + tsl],
                        st_d[1024:1088, t0 + u0 : t0 + u0 + tsl])
                    nc.scalar.dma_start(
                        wc[:, 0:8, u0 : u0 + tsl],
                        wct_d[0:1024, t0 + u0 : t0 + u0 + tsl].rearrange(
                            "(j p) t -> p j t", p=P),
                    )
                    nc.scalar.dma_start(
                        wc[0:64, 8, u0 : u0 + tsl],
                        wct_d[1024:1088, t0 + u0 : t0 + u0 + tsl])
                nc.scalar.dma_start(lt[:], latt_d[:, t0 : t0 + sz])
                xr = xt[:]
                if split_add:
                    for g in range(ng):
                        sl = slice(g * P, (g + 1) * P)
                        nc.vector.tensor_add(xr[:, :, sl], xt[:, :, sl], wc[:, :, sl])
                else:
                    nc.vector.tensor_add(xr, xt[:], wc[:])
                ltr = lt[:]

                for g in range(ng):
                    ts0 = g * P
                    out_sb = outpool.tile([P, D_MODEL], out_dt, tag="out")
                    # j-outer / h-inner: one stationary load per X^T chunk,
                    # both 512-col halves of W_read consumed per load
                    po = [mm_pool.tile([P, 512], F32, tag="mm", name=f"po{h}")
                          for h in range(2)]
                    for j, (r0, rw) in enumerate(R_CHUNKS):
                        for h in range(2):
                            nc.tensor.matmul(
                                po[h][:],
                                xr[0:rw, j, ts0 : ts0 + P],
                                wr_tiles[j][0:rw, h * 512 : h * 512 + 512],
                                start=(j == 0),
                                stop=(j == NJ - 1),
                            )
                    for h in range(2):
                        nc.vector.tensor_copy(out_sb[:, h * 512 : h * 512 + 512],
                                              po[h][:])

                    k_sb = outpool.tile([P, D_MODEL], out_dt, tag="k")
                    v_sb = outpool.tile([P, D_MODEL], out_dt, tag="v")
                    for h in range(2):
                        n0 = h * 512
                        pk = mm_pool.tile([P, 512], F32, tag="mm")
                        nc.tensor.matmul(
                            pk[:], ltr[:, ts0 : ts0 + P], wk_t[:, n0 : n0 + 512],
                            start=True, stop=True,
                        )
                        nc.scalar.copy(k_sb[:, n0 : n0 + 512], pk[:])
                        pv = mm_pool.tile([P, 512], F32, tag="mm")
                        nc.tensor.matmul(
                            pv[:], ltr[:, ts0 : ts0 + P], wv_t[:, n0 : n0 + 512],
                            start=True, stop=True,
                        )
                        nc.scalar.copy(v_sb[:, n0 : n0 + 512], pv[:])

                    eng_a, eng_b = (nc.scalar, nc.sync) if g % 2 == 0 else (nc.sync, nc.scalar)
                    row0 = t0 + ts0
                    eng_a.dma_start(out_d[row0 : row0 + P, :], out_sb[:])
                    eng_b.dma_start(k_d[row0 : row0 + P, :], k_sb[:])
                    eng_a.dma_start(v_d[row0 : row0 + P, :], v_sb[:])

    nc.compile()
    return nc


def _get_nc(**kw):
    key = tuple(sorted(kw.items()))
    if key not in _NC_CACHE:
        _NC_CACHE[key] = build_nc(**kw)
    return _NC_CACHE[key]


def make_in_maps(S, w_spoke, w_hub_priv, w_hub_shared, tag, W_read, cache, latent,
                 Wk, Wv):
    # host-side layout prep only (shard over batch, feature-major transposes,
    # bf16 downcast)
    bf = ml_dtypes.bfloat16
    wcat = np.concatenate(
        [np.asarray(w_spoke, np.float32), np.asarray(w_hub_priv, np.float32),
         np.asarray(w_hub_shared, np.float32), np.asarray(tag, np.float32)],
        axis=-1,
    )
    sT = np.ascontiguousarray(np.asarray(S, np.float32).transpose(0, 2, 1)).astype(bf)
    wcT = np.ascontiguousarray(wcat.transpose(0, 2, 1)).astype(bf)
    latT = np.ascontiguousarray(
        np.asarray(latent, np.float32).transpose(0, 2, 1)).astype(bf)
    wrt = np.ascontiguousarray(np.asarray(W_read, np.float32).T).astype(bf)
    wkt = np.ascontiguousarray(np.asarray(Wk, np.float32).T).astype(bf)
    wvt = np.ascontiguousarray(np.asarray(Wv, np.float32).T).astype(bf)
    return [
        {"st": sT[i], "wct": wcT[i], "latt": latT[i],
         "wrt": wrt, "wkt": wkt, "wvt": wvt}
        for i in range(N_CORES)
    ]


def kernel(S, w_spoke, w_hub_priv, w_hub_shared, tag, W_read, cache, latent, Wk, Wv,
           **build_kw):
    in_maps = make_in_maps(S, w_spoke, w_hub_priv, w_hub_shared, tag, W_read, cache,
                           latent, Wk, Wv)
    nc = _get_nc(**build_kw)
    res = run_bass_kernel_spmd(nc, in_maps, list(range(N_CORES)))
    out = np.stack([res.results[i]["out"].astype(np.float32) for i in range(N_CORES)])
    k = np.stack([res.results[i]["k"].astype(np.float32) for i in range(N_CORES)])
    v = np.stack([res.results[i]["v"].astype(np.float32) for i in range(N_CORES)])
    return (out, k, v)
